# revision 11
# baseline (speedup 1.0000x reference)
"""CenterLoss kernel for Trainium2 (8 NeuronCores, data-parallel over W).

loss = sum_{n,c,w} act[n,c,w] * dist[n,c,w],  clipped at 1e-6, where
  dist[n,c,w] = ||x[n,:,w] - ctr[:,c]||^2 = x2[n,w] - 2*xc[n,c,w] + c2[c]

Sharding: each core takes a W/8 = 2048-column slice (all 16 n, all 80 c).
Both x and act are pre-cast to bf16 on the host, so every DMA is a plain
(cast-free) HWDGE transfer and on-device HBM traffic is halved: 9.2 MiB
per core vs 18.4 fp32.  The SDMA engines are bus-limited at ~27.5 GB/s of
read+write bytes each, so halving bytes on both sides is the main lever.

Per-core structure:
  - x tiles: one [128, 2048] bf16 tile per n holding [x ; x^2] stacked
    along the contraction dim (x^2 squared on ScalarE into the other
    half; layout flipped for odd n so the two DMA halves alternate SBUF
    port halves).
  - act tiles: [128, 2048] bf16 tiles covering 128 consecutive (n,c)
    rows of the [16*80, 2048] act slice - full-width DMAs and full-width
    DVE ops.  The 80-vs-128 misalignment is handled per tile by 1-3
    "n-runs": each run gets its own K=128 matmul with a column slice of
    the [-2c ; 1] weights, writing its partition range of the PSUM tile.
  - One fused DVE scalar_tensor_tensor per [128, 1024] PSUM tile computes
    (dist' + c2) * act and row-sums into a column of a [128, 20]
    accumulator (c2 mapped per (tile, partition) on the host).
  - Tail: reduce the accumulator, ones^T @ racc -> [1,1] on PE.
    Host sums the 8 per-core partials and applies the clip.
  - All tile pools are sized so no buffer is ever recycled: every DMA
    issues with no waits and the HWDGE ring streams back-to-back.
"""

import os
import sys

import numpy as np

for _p in ("/opt/trn_rl_repo",):
    if _p not in sys.path and os.path.isdir(_p):
        sys.path.insert(0, _p)

N, D, C, W = 16, 64, 80, 16384
NCORES = 8
WC = W // NCORES  # 2048 columns per core
NT = (N * C) // 128  # 10 act tiles of 128 rows
SUB = 1024  # free-dim per PSUM tile / fused DVE op
MMN = 512  # matmul free dim (one PSUM bank)
NACC = 2 * NT  # 20 accumulator columns

_CACHE = {}


def _runs():
    """Per act tile t: list of (p0, n, c0, L) n-runs covering its 128 rows."""
    out = []
    for t in range(NT):
        r0 = t * 128
        runs = []
        r = r0
        while r < r0 + 128:
            n, c0 = divmod(r, C)
            L = min(C - c0, r0 + 128 - r)
            runs.append((r - r0, n, c0, L))
            r += L
        out.append(runs)
    return out


def _build_bass():
    import concourse.bacc as bacc
    import concourse.tile as tile
    from concourse import mybir

    fp32 = mybir.dt.float32
    bf16 = mybir.dt.bfloat16
    Alu = mybir.AluOpType

    nc = bacc.Bacc("TRN2", target_bir_lowering=False)

    from contextlib import ExitStack

    RUNS = _runs()
    NRUNS = sum(len(r) for r in RUNS)  # 24

    xs = nc.dram_tensor("xs", [N * D, WC], bf16, kind="ExternalInput")
    acts = nc.dram_tensor("acts", [N * C, WC], bf16, kind="ExternalInput")
    # wR: one zero-padded [128, 128] weight block per n-run (PSUM base
    # partition must be quadrant-aligned, so every matmul spans all 128
    # output partitions and runs accumulate into the same bank).
    wR = nc.dram_tensor("wR", [128, NRUNS * 128], bf16, kind="ExternalInput")
    wtc = nc.dram_tensor("wtc", [128, NT + 1], fp32, kind="ExternalInput")
    out = nc.dram_tensor("out", [1, 1], fp32, kind="ExternalOutput")

    # squares offloaded from ScalarE to DVE for these n (load balance)
    DVE_SQ = {5, 9, 13}

    with tile.TileContext(nc) as tc, ExitStack() as ctx:
        consts = ctx.enter_context(tc.tile_pool(name="consts", bufs=1))
        xpool = ctx.enter_context(tc.tile_pool(name="xpool", bufs=N))
        apool = ctx.enter_context(tc.tile_pool(name="apool", bufs=2))
        appool = ctx.enter_context(tc.tile_pool(name="appool", bufs=(NT - 2) // 2))
        spool = ctx.enter_context(tc.tile_pool(name="spool", bufs=2))
        rpool = ctx.enter_context(tc.tile_pool(name="rpool", bufs=1))
        opool = ctx.enter_context(tc.tile_pool(name="opool", bufs=1))
        pdist = ctx.enter_context(tc.tile_pool(name="pdist", bufs=3, space="PSUM"))
        psmall = ctx.enter_context(tc.tile_pool(name="psmall", bufs=1, space="PSUM"))

        racc_all = rpool.tile([128, NACC], fp32)

        xx = {}

        def load_x(n):
            # SWDGE (gpsimd) so descriptor generation runs in parallel with
            # the act loads' HWDGE generation on the sync sequencer.
            t_ = xpool.tile([128, WC], bf16, tag="xx")
            flip = n % 2 == 1
            # First two n: DMA+square in col slices so the first matmuls
            # can start sooner (pipeline fill).
            nslc = 2 if n < 2 else 1
            for j in range(nslc):
                s0, s1 = j * (WC // nslc), (j + 1) * (WC // nslc)
                if not flip:
                    nc.gpsimd.dma_start(
                        out=t_[0:D, s0:s1], in_=xs[n * D : (n + 1) * D, s0:s1]
                    )
                    dst, src = t_[D : 2 * D, s0:s1], t_[0:D, s0:s1]
                else:
                    nc.gpsimd.dma_start(
                        out=t_[D : 2 * D, s0:s1], in_=xs[n * D : (n + 1) * D, s0:s1]
                    )
                    dst, src = t_[0:D, s0:s1], t_[D : 2 * D, s0:s1]
                if n in DVE_SQ:
                    nc.vector.tensor_mul(dst, src, src)
                else:
                    nc.scalar.square(out=dst, in_=src)
            xx[n] = t_

        at = {}  # t -> (tile, col offset of t within the tile)

        def load_act(t):
            if t in (0, NT - 1):
                # single tile, 2 col slices (fast pipeline fill / drain)
                a_ = apool.tile([128, WC], bf16, tag="at")
                for j in range(2):
                    s0, s1 = j * (WC // 2), (j + 1) * (WC // 2)
                    nc.sync.dma_start(
                        out=a_[:, s0:s1], in_=acts[t * 128 : (t + 1) * 128, s0:s1]
                    )
                at[t] = (a_, 0)
            else:
                # paired tiles (t, t+1) in one [128, 2*WC] DMA: halves the
                # HWDGE dispatch count on the sync sequencer
                a_ = appool.tile([128, 2 * WC], bf16, tag="atp")
                src = acts[t * 128 : (t + 2) * 128, :].rearrange(
                    "(j p) w -> p j w", j=2
                )
                nc.sync.dma_start(out=a_[:], in_=src)
                at[t] = (a_, 0)
                at[t + 1] = (a_, WC)

        iacc = 0
        ri0 = 0  # index of tile t's first run within the flat run list
        for t in range(NT):
            for (_p0, n, _c0, _L) in RUNS[t]:
                if n not in xx:
                    load_x(n)
            if t not in at:
                load_act(t)
            if t == 0:
                # consts after the first x/act dispatches: they are only
                # needed once the first squares complete
                wR_t = consts.tile([128, NRUNS * 128], bf16)
                nc.sync.dma_start(out=wR_t[:], in_=wR[:, :])
                wtc_t = consts.tile([128, NT + 1], fp32)
                nc.sync.dma_start(out=wtc_t[:], in_=wtc[:, :])
            nrun = len(RUNS[t])
            for si in range(WC // SUB):
                pd = pdist.tile([128, SUB], fp32, tag="pd")
                for mi in range(SUB // MMN):
                    col = si * SUB + mi * MMN
                    for k, (_p0, n, _c0, _L) in enumerate(RUNS[t]):
                        ri = ri0 + k
                        nc.tensor.matmul(
                            pd[:, mi * MMN : (mi + 1) * MMN],
                            wR_t[:, ri * 128 : (ri + 1) * 128],
                            xx[n][:, col : col + MMN],
                            start=(k == 0),
                            stop=(k == nrun - 1),
                        )
                a_, aoff = at[t]
                scr = spool.tile([128, SUB], fp32, tag="scr")
                nc.vector.scalar_tensor_tensor(
                    out=scr[:],
                    in0=pd[:],
                    scalar=wtc_t[:, t : t + 1],
                    in1=a_[:, aoff + si * SUB : aoff + (si + 1) * SUB],
                    op0=Alu.add,
                    op1=Alu.mult,
                    accum_out=racc_all[:, iacc : iacc + 1],
                )
                iacc += 1
            ri0 += nrun

        # loss_core = ones^T @ (row-sums of racc_all)
        racc = opool.tile([128, 1], fp32, tag="racc")
        nc.vector.tensor_reduce(
            out=racc[:], in_=racc_all[:], axis=mybir.AxisListType.X, op=Alu.add
        )
        pfin = psmall.tile([1, 1], fp32)
        nc.tensor.matmul(pfin[:], wtc_t[:, NT : NT + 1], racc[:], start=True, stop=True)
        osb = opool.tile([1, 1], fp32, tag="osb")
        nc.vector.tensor_copy(osb[:], pfin[:])
        nc.sync.dma_start(out=out[:, :], in_=osb[:])

    nc.compile()
    return nc


def _get_nc():
    if "nc" not in _CACHE:
        _CACHE["nc"] = _build_bass()
    return _CACHE["nc"]


def prepare_in_maps(x, c, act):
    import ml_dtypes

    bf16 = ml_dtypes.bfloat16
    x = np.ascontiguousarray(np.asarray(x), dtype=np.float32)
    c = np.ascontiguousarray(np.asarray(c), dtype=np.float32)
    act = np.ascontiguousarray(np.asarray(act), dtype=np.float32)
    assert x.shape == (N, D, W) and c.shape == (D, C) and act.shape == (N, C, W)

    xb = x.astype(bf16)
    ab = act.astype(bf16)
    c2 = np.sum(c * c, axis=0, dtype=np.float32)  # [C]
    ones_dc = np.ones((D, C), dtype=np.float32)
    wA = np.concatenate([-2.0 * c, ones_dc], axis=0)  # [128, C], even n
    wB = np.concatenate([ones_dc, -2.0 * c], axis=0)  # [128, C], odd n
    # Zero-padded per-run weight blocks (see wR comment in _build_bass).
    RUNS = _runs()
    blocks = []
    for t in range(NT):
        for (p0, n, c0, L) in RUNS[t]:
            blk = np.zeros((128, 128), dtype=np.float32)
            w_n = wB if n % 2 else wA
            blk[:, p0 : p0 + L] = w_n[:, c0 : c0 + L]
            blocks.append(blk)
    wRh = np.ascontiguousarray(np.concatenate(blocks, axis=1), dtype=bf16)
    # wtc[p, t] = c2[(t*128+p) % C] for act tile t; last col = ones.
    wtc = np.ones((128, NT + 1), dtype=np.float32)
    p = np.arange(128)
    for t in range(NT):
        wtc[:, t] = c2[(t * 128 + p) % C]
    wtc = np.ascontiguousarray(wtc)

    in_maps = []
    for k in range(NCORES):
        sl = slice(k * WC, (k + 1) * WC)
        in_maps.append(
            {
                "xs": np.ascontiguousarray(xb[:, :, sl]).reshape(N * D, WC),
                "acts": np.ascontiguousarray(ab[:, :, sl]).reshape(N * C, WC),
                "wR": wRh,
                "wtc": wtc,
            }
        )
    return in_maps


def kernel(x, c, act):
    from concourse.bass_utils import run_bass_kernel_spmd

    in_maps = prepare_in_maps(x, c, act)
    res = run_bass_kernel_spmd(_get_nc(), in_maps, core_ids=list(range(NCORES)))
    total = np.float32(0.0)
    for r in res.results:
        total = np.float32(total + np.float32(r["out"][0, 0]))
    return np.maximum(np.float32(total), np.float32(1e-6))


# revision 13
# speedup vs baseline: 1.1357x; 1.1357x over previous
"""CenterLoss kernel for Trainium2 (8 NeuronCores, data-parallel over W).

loss = sum_{n,c,w} act[n,c,w] * dist[n,c,w],  clipped at 1e-6, where
  dist[n,c,w] = ||x[n,:,w] - ctr[:,c]||^2 = x2[n,w] - 2*xc[n,c,w] + c2[c]

Sharding: each core takes a W/8 = 2048-column slice (all 16 n, all 80 c).
Both x and act are pre-cast to bf16 on the host, so every DMA is a plain
(cast-free) HWDGE transfer and on-device HBM traffic is halved: 9.2 MiB
per core vs 18.4 fp32.  The SDMA engines are bus-limited at ~27.5 GB/s of
read+write bytes each, so halving bytes on both sides is the main lever.

Per-core structure:
  - x tiles: one [128, 2048] bf16 tile per n holding [x ; x^2] stacked
    along the contraction dim (x^2 squared on ScalarE into the other
    half; layout flipped for odd n so the two DMA halves alternate SBUF
    port halves).
  - act tiles: [128, 2048] bf16 tiles covering 128 consecutive (n,c)
    rows of the [16*80, 2048] act slice - full-width DMAs and full-width
    DVE ops.  The 80-vs-128 misalignment is handled per tile by 1-3
    "n-runs": each run gets its own K=128 matmul with a column slice of
    the [-2c ; 1] weights, writing its partition range of the PSUM tile.
  - One fused DVE scalar_tensor_tensor per [128, 1024] PSUM tile computes
    (dist' + c2) * act and row-sums into a column of a [128, 20]
    accumulator (c2 mapped per (tile, partition) on the host).
  - Tail: reduce the accumulator, ones^T @ racc -> [1,1] on PE.
    Host sums the 8 per-core partials and applies the clip.
  - All tile pools are sized so no buffer is ever recycled: every DMA
    issues with no waits and the HWDGE ring streams back-to-back.
"""

import os
import sys

import numpy as np

for _p in ("/opt/trn_rl_repo",):
    if _p not in sys.path and os.path.isdir(_p):
        sys.path.insert(0, _p)

N, D, C, W = 16, 64, 80, 16384
NCORES = 8
WC = W // NCORES  # 2048 columns per core
NT = (N * C) // 128  # 10 act tiles of 128 rows
SUB = 1024  # free-dim per PSUM tile / fused DVE op
MMN = 512  # matmul free dim (one PSUM bank)
NACC = 2 * NT  # 20 accumulator columns

_CACHE = {}


def _runs():
    """Per act tile t: list of (p0, n, c0, L) n-runs covering its 128 rows."""
    out = []
    for t in range(NT):
        r0 = t * 128
        runs = []
        r = r0
        while r < r0 + 128:
            n, c0 = divmod(r, C)
            L = min(C - c0, r0 + 128 - r)
            runs.append((r - r0, n, c0, L))
            r += L
        out.append(runs)
    return out


def _build_bass():
    import concourse.bacc as bacc
    import concourse.tile as tile
    from concourse import mybir

    fp32 = mybir.dt.float32
    bf16 = mybir.dt.bfloat16
    Alu = mybir.AluOpType

    nc = bacc.Bacc("TRN2", target_bir_lowering=False)

    from contextlib import ExitStack

    RUNS = _runs()
    NRUNS = sum(len(r) for r in RUNS)  # 24

    xs = nc.dram_tensor("xs", [N * D, WC], bf16, kind="ExternalInput")
    acts = nc.dram_tensor("acts", [N * C, WC], bf16, kind="ExternalInput")
    # wR: one zero-padded [128, 128] weight block per n-run (PSUM base
    # partition must be quadrant-aligned, so every matmul spans all 128
    # output partitions and runs accumulate into the same bank).
    wR = nc.dram_tensor("wR", [128, NRUNS * 128], bf16, kind="ExternalInput")
    wtc = nc.dram_tensor("wtc", [128, NT + 1], fp32, kind="ExternalInput")
    out = nc.dram_tensor("out", [1, 1], fp32, kind="ExternalOutput")

    # squares offloaded from ScalarE to DVE for these n (load balance)
    DVE_SQ = {5, 9, 13}

    with tile.TileContext(nc) as tc, ExitStack() as ctx:
        consts = ctx.enter_context(tc.tile_pool(name="consts", bufs=1))
        xpool = ctx.enter_context(tc.tile_pool(name="xpool", bufs=N))
        apool = ctx.enter_context(tc.tile_pool(name="apool", bufs=2))
        appool = ctx.enter_context(tc.tile_pool(name="appool", bufs=(NT - 2) // 2))
        spool = ctx.enter_context(tc.tile_pool(name="spool", bufs=2))
        rpool = ctx.enter_context(tc.tile_pool(name="rpool", bufs=1))
        opool = ctx.enter_context(tc.tile_pool(name="opool", bufs=1))
        pdist = ctx.enter_context(tc.tile_pool(name="pdist", bufs=3, space="PSUM"))
        psmall = ctx.enter_context(tc.tile_pool(name="psmall", bufs=1, space="PSUM"))

        racc_all = rpool.tile([128, NACC], fp32)

        xx = {}

        def load_x(n):
            t_ = xpool.tile([128, WC], bf16, tag="xx")
            flip = n % 2 == 1
            # First two n: DMA+square in col slices so the first matmuls
            # can start sooner (pipeline fill).
            nslc = 2 if n < 2 else 1
            for j in range(nslc):
                s0, s1 = j * (WC // nslc), (j + 1) * (WC // nslc)
                if not flip:
                    nc.sync.dma_start(
                        out=t_[0:D, s0:s1], in_=xs[n * D : (n + 1) * D, s0:s1]
                    )
                    dst, src = t_[D : 2 * D, s0:s1], t_[0:D, s0:s1]
                else:
                    nc.sync.dma_start(
                        out=t_[D : 2 * D, s0:s1], in_=xs[n * D : (n + 1) * D, s0:s1]
                    )
                    dst, src = t_[0:D, s0:s1], t_[D : 2 * D, s0:s1]
                if n in DVE_SQ:
                    nc.vector.tensor_mul(dst, src, src)
                else:
                    nc.scalar.square(out=dst, in_=src)
            xx[n] = t_

        at = {}  # t -> (tile, col offset of t within the tile)

        def load_act(t):
            if t in (0, NT - 1):
                # single tile, col-sliced (fast pipeline fill / drain)
                a_ = apool.tile([128, WC], bf16, tag="at")
                nslc = 4 if t == NT - 1 else 2
                for j in range(nslc):
                    s0, s1 = j * (WC // nslc), (j + 1) * (WC // nslc)
                    nc.sync.dma_start(
                        out=a_[:, s0:s1], in_=acts[t * 128 : (t + 1) * 128, s0:s1]
                    )
                at[t] = (a_, 0)
            else:
                # paired tiles (t, t+1) in one [128, 2*WC] DMA: halves the
                # HWDGE dispatch count on the sync sequencer
                a_ = appool.tile([128, 2 * WC], bf16, tag="atp")
                src = acts[t * 128 : (t + 2) * 128, :].rearrange(
                    "(j p) w -> p j w", j=2
                )
                nc.sync.dma_start(out=a_[:], in_=src)
                at[t] = (a_, 0)
                at[t + 1] = (a_, WC)

        iacc = 0
        ri0 = 0  # index of tile t's first run within the flat run list
        for t in range(NT):
            for (_p0, n, _c0, _L) in RUNS[t]:
                if n not in xx:
                    load_x(n)
            if t not in at:
                load_act(t)
            if t == 0:
                # consts after the first x/act dispatches: they are only
                # needed once the first squares complete
                wR_t = consts.tile([128, NRUNS * 128], bf16)
                nc.sync.dma_start(out=wR_t[:], in_=wR[:, :])
                wtc_t = consts.tile([128, NT + 1], fp32)
                nc.sync.dma_start(out=wtc_t[:], in_=wtc[:, :])
            nrun = len(RUNS[t])
            for si in range(WC // SUB):
                pd = pdist.tile([128, SUB], fp32, tag="pd")
                for mi in range(SUB // MMN):
                    col = si * SUB + mi * MMN
                    for k, (_p0, n, _c0, _L) in enumerate(RUNS[t]):
                        ri = ri0 + k
                        nc.tensor.matmul(
                            pd[:, mi * MMN : (mi + 1) * MMN],
                            wR_t[:, ri * 128 : (ri + 1) * 128],
                            xx[n][:, col : col + MMN],
                            start=(k == 0),
                            stop=(k == nrun - 1),
                        )
                a_, aoff = at[t]
                scr = spool.tile([128, SUB], fp32, tag="scr")
                nc.vector.scalar_tensor_tensor(
                    out=scr[:],
                    in0=pd[:],
                    scalar=wtc_t[:, t : t + 1],
                    in1=a_[:, aoff + si * SUB : aoff + (si + 1) * SUB],
                    op0=Alu.add,
                    op1=Alu.mult,
                    accum_out=racc_all[:, iacc : iacc + 1],
                )
                iacc += 1
            ri0 += nrun

        # loss_core = ones^T @ (row-sums of racc_all)
        racc = opool.tile([128, 1], fp32, tag="racc")
        nc.vector.tensor_reduce(
            out=racc[:], in_=racc_all[:], axis=mybir.AxisListType.X, op=Alu.add
        )
        pfin = psmall.tile([1, 1], fp32)
        nc.tensor.matmul(pfin[:], wtc_t[:, NT : NT + 1], racc[:], start=True, stop=True)
        osb = opool.tile([1, 1], fp32, tag="osb")
        nc.vector.tensor_copy(osb[:], pfin[:])
        nc.sync.dma_start(out=out[:, :], in_=osb[:])

    nc.compile()
    return nc


def _get_nc():
    if "nc" not in _CACHE:
        _CACHE["nc"] = _build_bass()
    return _CACHE["nc"]


def prepare_in_maps(x, c, act):
    import ml_dtypes

    bf16 = ml_dtypes.bfloat16
    x = np.ascontiguousarray(np.asarray(x), dtype=np.float32)
    c = np.ascontiguousarray(np.asarray(c), dtype=np.float32)
    act = np.ascontiguousarray(np.asarray(act), dtype=np.float32)
    assert x.shape == (N, D, W) and c.shape == (D, C) and act.shape == (N, C, W)

    xb = x.astype(bf16)
    ab = act.astype(bf16)
    c2 = np.sum(c * c, axis=0, dtype=np.float32)  # [C]
    ones_dc = np.ones((D, C), dtype=np.float32)
    wA = np.concatenate([-2.0 * c, ones_dc], axis=0)  # [128, C], even n
    wB = np.concatenate([ones_dc, -2.0 * c], axis=0)  # [128, C], odd n
    # Zero-padded per-run weight blocks (see wR comment in _build_bass).
    RUNS = _runs()
    blocks = []
    for t in range(NT):
        for (p0, n, c0, L) in RUNS[t]:
            blk = np.zeros((128, 128), dtype=np.float32)
            w_n = wB if n % 2 else wA
            blk[:, p0 : p0 + L] = w_n[:, c0 : c0 + L]
            blocks.append(blk)
    wRh = np.ascontiguousarray(np.concatenate(blocks, axis=1), dtype=bf16)
    # wtc[p, t] = c2[(t*128+p) % C] for act tile t; last col = ones.
    wtc = np.ones((128, NT + 1), dtype=np.float32)
    p = np.arange(128)
    for t in range(NT):
        wtc[:, t] = c2[(t * 128 + p) % C]
    wtc = np.ascontiguousarray(wtc)

    in_maps = []
    for k in range(NCORES):
        sl = slice(k * WC, (k + 1) * WC)
        in_maps.append(
            {
                "xs": np.ascontiguousarray(xb[:, :, sl]).reshape(N * D, WC),
                "acts": np.ascontiguousarray(ab[:, :, sl]).reshape(N * C, WC),
                "wR": wRh,
                "wtc": wtc,
            }
        )
    return in_maps


def kernel(x, c, act):
    from concourse.bass_utils import run_bass_kernel_spmd

    in_maps = prepare_in_maps(x, c, act)
    res = run_bass_kernel_spmd(_get_nc(), in_maps, core_ids=list(range(NCORES)))
    total = np.float32(0.0)
    for r in res.results:
        total = np.float32(total + np.float32(r["out"][0, 0]))
    return np.maximum(np.float32(total), np.float32(1e-6))


# revision 15
# speedup vs baseline: 1.2062x; 1.0620x over previous
"""CenterLoss kernel for Trainium2 (8 NeuronCores, data-parallel over W).

loss = sum_{n,c,w} act[n,c,w] * dist[n,c,w],  clipped at 1e-6, where
  dist[n,c,w] = ||x[n,:,w] - ctr[:,c]||^2 = x2[n,w] - 2*xc[n,c,w] + c2[c]

Sharding: each core takes a W/8 = 2048-column slice (all 16 n, all 80 c).
Both x and act are pre-cast to bf16 on the host, so every DMA is a plain
(cast-free) HWDGE transfer and on-device HBM traffic is halved: 9.2 MiB
per core vs 18.4 fp32.  The SDMA engines are bus-limited at ~27.5 GB/s of
read+write bytes each, so halving bytes on both sides is the main lever.

Per-core structure:
  - x tiles: one [128, 2048] bf16 tile per n holding [x ; x^2] stacked
    along the contraction dim (x^2 squared on ScalarE into the other
    half; layout flipped for odd n so the two DMA halves alternate SBUF
    port halves).
  - act tiles: [128, 2048] bf16 tiles covering 128 consecutive (n,c)
    rows of the [16*80, 2048] act slice - full-width DMAs and full-width
    DVE ops.  The 80-vs-128 misalignment is handled per tile by 1-3
    "n-runs": each run gets its own K=128 matmul with a column slice of
    the [-2c ; 1] weights, writing its partition range of the PSUM tile.
  - One fused DVE scalar_tensor_tensor per [128, 1024] PSUM tile computes
    (dist' + c2) * act and row-sums into a column of a [128, 20]
    accumulator (c2 mapped per (tile, partition) on the host).
  - Tail: reduce the accumulator, ones^T @ racc -> [1,1] on PE.
    Host sums the 8 per-core partials and applies the clip.
  - All tile pools are sized so no buffer is ever recycled: every DMA
    issues with no waits and the HWDGE ring streams back-to-back.
"""

import os
import sys

import numpy as np

for _p in ("/opt/trn_rl_repo",):
    if _p not in sys.path and os.path.isdir(_p):
        sys.path.insert(0, _p)

N, D, C, W = 16, 64, 80, 16384
NCORES = 8
WC = W // NCORES  # 2048 columns per core
NT = (N * C) // 128  # 10 act tiles of 128 rows
SUB = 1024  # free-dim per PSUM tile / fused DVE op
MMN = 512  # matmul free dim (one PSUM bank)
NACC = 2 * NT  # 20 accumulator columns

_CACHE = {}


def _runs():
    """Per act tile t: list of (p0, n, c0, L) n-runs covering its 128 rows."""
    out = []
    for t in range(NT):
        r0 = t * 128
        runs = []
        r = r0
        while r < r0 + 128:
            n, c0 = divmod(r, C)
            L = min(C - c0, r0 + 128 - r)
            runs.append((r - r0, n, c0, L))
            r += L
        out.append(runs)
    return out


def _build_bass():
    import concourse.bacc as bacc
    import concourse.tile as tile
    from concourse import mybir

    fp32 = mybir.dt.float32
    bf16 = mybir.dt.bfloat16
    Alu = mybir.AluOpType

    nc = bacc.Bacc("TRN2", target_bir_lowering=False)

    from contextlib import ExitStack

    RUNS = _runs()
    NRUNS = sum(len(r) for r in RUNS)  # 24

    xs = nc.dram_tensor("xs", [N * D, WC], bf16, kind="ExternalInput")
    acts = nc.dram_tensor("acts", [N * C, WC], bf16, kind="ExternalInput")
    # wR: one zero-padded [128, 128] weight block per n-run (PSUM base
    # partition must be quadrant-aligned, so every matmul spans all 128
    # output partitions and runs accumulate into the same bank).
    wR = nc.dram_tensor("wR", [128, NRUNS * 128], bf16, kind="ExternalInput")
    wtc = nc.dram_tensor("wtc", [128, NT + 1], fp32, kind="ExternalInput")
    out = nc.dram_tensor("out", [1, 1], fp32, kind="ExternalOutput")

    # squares offloaded from ScalarE to DVE for these n (load balance;
    # mid-stream n only, so DVE is free near the drain)
    DVE_SQ = {5, 7, 9}

    with tile.TileContext(nc) as tc, ExitStack() as ctx:
        consts = ctx.enter_context(tc.tile_pool(name="consts", bufs=1))
        xpool = ctx.enter_context(tc.tile_pool(name="xpool", bufs=N))
        apool = ctx.enter_context(tc.tile_pool(name="apool", bufs=2))
        appool = ctx.enter_context(tc.tile_pool(name="appool", bufs=(NT - 2) // 2))
        spool = ctx.enter_context(tc.tile_pool(name="spool", bufs=2))
        rpool = ctx.enter_context(tc.tile_pool(name="rpool", bufs=1))
        opool = ctx.enter_context(tc.tile_pool(name="opool", bufs=1))
        pdist = ctx.enter_context(tc.tile_pool(name="pdist", bufs=3, space="PSUM"))
        psmall = ctx.enter_context(tc.tile_pool(name="psmall", bufs=1, space="PSUM"))

        racc_all = rpool.tile([128, NACC], fp32)

        xx = {}

        def load_x(n):
            t_ = xpool.tile([128, WC], bf16, tag="xx")
            flip = n % 2 == 1
            # First two n: DMA+square in col slices so the first matmuls
            # can start sooner (pipeline fill).
            nslc = 2 if n < 2 else 1
            for j in range(nslc):
                s0, s1 = j * (WC // nslc), (j + 1) * (WC // nslc)
                if not flip:
                    nc.sync.dma_start(
                        out=t_[0:D, s0:s1], in_=xs[n * D : (n + 1) * D, s0:s1]
                    )
                    dst, src = t_[D : 2 * D, s0:s1], t_[0:D, s0:s1]
                else:
                    nc.sync.dma_start(
                        out=t_[D : 2 * D, s0:s1], in_=xs[n * D : (n + 1) * D, s0:s1]
                    )
                    dst, src = t_[0:D, s0:s1], t_[D : 2 * D, s0:s1]
                if n in DVE_SQ:
                    nc.vector.tensor_mul(dst, src, src)
                else:
                    nc.scalar.square(out=dst, in_=src)
            xx[n] = t_

        at = {}  # t -> (tile, col offset of t within the tile)

        def load_act(t):
            if t in (0, NT - 1):
                # single tile, col-sliced (fast pipeline fill / drain)
                a_ = apool.tile([128, WC], bf16, tag="at")
                nslc = 4 if t == NT - 1 else 2
                for j in range(nslc):
                    s0, s1 = j * (WC // nslc), (j + 1) * (WC // nslc)
                    nc.sync.dma_start(
                        out=a_[:, s0:s1], in_=acts[t * 128 : (t + 1) * 128, s0:s1]
                    )
                at[t] = (a_, 0)
            else:
                # paired tiles (t, t+1) in one [128, 2*WC] DMA: halves the
                # HWDGE dispatch count on the sync sequencer
                a_ = appool.tile([128, 2 * WC], bf16, tag="atp")
                src = acts[t * 128 : (t + 2) * 128, :].rearrange(
                    "(j p) w -> p j w", j=2
                )
                nc.sync.dma_start(out=a_[:], in_=src)
                at[t] = (a_, 0)
                at[t + 1] = (a_, WC)

        iacc = 0
        ri0 = 0  # index of tile t's first run within the flat run list
        for t in range(NT):
            # x for this tile AND the next (prefetch): squares must be done
            # before the next tile's act bytes land, else they chain after
            # the last DMA and stretch the drain.
            for tt in (t, t + 1):
                if tt < NT:
                    for (_p0, n, _c0, _L) in RUNS[tt]:
                        if n not in xx:
                            load_x(n)
            if t not in at:
                load_act(t)
            if t == 0:
                # consts after the first x/act dispatches: they are only
                # needed once the first squares complete
                wR_t = consts.tile([128, NRUNS * 128], bf16)
                nc.sync.dma_start(out=wR_t[:], in_=wR[:, :])
                wtc_t = consts.tile([128, NT + 1], fp32)
                nc.sync.dma_start(out=wtc_t[:], in_=wtc[:, :])
            nrun = len(RUNS[t])
            for si in range(WC // SUB):
                pd = pdist.tile([128, SUB], fp32, tag="pd")
                for mi in range(SUB // MMN):
                    col = si * SUB + mi * MMN
                    for k, (_p0, n, _c0, _L) in enumerate(RUNS[t]):
                        ri = ri0 + k
                        nc.tensor.matmul(
                            pd[:, mi * MMN : (mi + 1) * MMN],
                            wR_t[:, ri * 128 : (ri + 1) * 128],
                            xx[n][:, col : col + MMN],
                            start=(k == 0),
                            stop=(k == nrun - 1),
                        )
                a_, aoff = at[t]
                scr = spool.tile([128, SUB], fp32, tag="scr")
                nc.vector.scalar_tensor_tensor(
                    out=scr[:],
                    in0=pd[:],
                    scalar=wtc_t[:, t : t + 1],
                    in1=a_[:, aoff + si * SUB : aoff + (si + 1) * SUB],
                    op0=Alu.add,
                    op1=Alu.mult,
                    accum_out=racc_all[:, iacc : iacc + 1],
                )
                iacc += 1
            ri0 += nrun

        # loss_core = ones^T @ (row-sums of racc_all)
        racc = opool.tile([128, 1], fp32, tag="racc")
        nc.vector.tensor_reduce(
            out=racc[:], in_=racc_all[:], axis=mybir.AxisListType.X, op=Alu.add
        )
        pfin = psmall.tile([1, 1], fp32)
        nc.tensor.matmul(pfin[:], wtc_t[:, NT : NT + 1], racc[:], start=True, stop=True)
        osb = opool.tile([1, 1], fp32, tag="osb")
        nc.vector.tensor_copy(osb[:], pfin[:])
        nc.sync.dma_start(out=out[:, :], in_=osb[:])

    nc.compile()
    return nc


def _get_nc():
    if "nc" not in _CACHE:
        _CACHE["nc"] = _build_bass()
    return _CACHE["nc"]


def prepare_in_maps(x, c, act):
    import ml_dtypes

    bf16 = ml_dtypes.bfloat16
    x = np.ascontiguousarray(np.asarray(x), dtype=np.float32)
    c = np.ascontiguousarray(np.asarray(c), dtype=np.float32)
    act = np.ascontiguousarray(np.asarray(act), dtype=np.float32)
    assert x.shape == (N, D, W) and c.shape == (D, C) and act.shape == (N, C, W)

    xb = x.astype(bf16)
    ab = act.astype(bf16)
    c2 = np.sum(c * c, axis=0, dtype=np.float32)  # [C]
    ones_dc = np.ones((D, C), dtype=np.float32)
    wA = np.concatenate([-2.0 * c, ones_dc], axis=0)  # [128, C], even n
    wB = np.concatenate([ones_dc, -2.0 * c], axis=0)  # [128, C], odd n
    # Zero-padded per-run weight blocks (see wR comment in _build_bass).
    RUNS = _runs()
    blocks = []
    for t in range(NT):
        for (p0, n, c0, L) in RUNS[t]:
            blk = np.zeros((128, 128), dtype=np.float32)
            w_n = wB if n % 2 else wA
            blk[:, p0 : p0 + L] = w_n[:, c0 : c0 + L]
            blocks.append(blk)
    wRh = np.ascontiguousarray(np.concatenate(blocks, axis=1), dtype=bf16)
    # wtc[p, t] = c2[(t*128+p) % C] for act tile t; last col = ones.
    wtc = np.ones((128, NT + 1), dtype=np.float32)
    p = np.arange(128)
    for t in range(NT):
        wtc[:, t] = c2[(t * 128 + p) % C]
    wtc = np.ascontiguousarray(wtc)

    in_maps = []
    for k in range(NCORES):
        sl = slice(k * WC, (k + 1) * WC)
        in_maps.append(
            {
                "xs": np.ascontiguousarray(xb[:, :, sl]).reshape(N * D, WC),
                "acts": np.ascontiguousarray(ab[:, :, sl]).reshape(N * C, WC),
                "wR": wRh,
                "wtc": wtc,
            }
        )
    return in_maps


def kernel(x, c, act):
    from concourse.bass_utils import run_bass_kernel_spmd

    in_maps = prepare_in_maps(x, c, act)
    res = run_bass_kernel_spmd(_get_nc(), in_maps, core_ids=list(range(NCORES)))
    total = np.float32(0.0)
    for r in res.results:
        total = np.float32(total + np.float32(r["out"][0, 0]))
    return np.maximum(np.float32(total), np.float32(1e-6))


# revision 16
# speedup vs baseline: 1.4421x; 1.1956x over previous
"""CenterLoss kernel for Trainium2 (8 NeuronCores, data-parallel over W).

loss = sum_{n,c,w} act[n,c,w] * dist[n,c,w],  clipped at 1e-6, where
  dist[n,c,w] = ||x[n,:,w] - ctr[:,c]||^2 = x2[n,w] - 2*xc[n,c,w] + c2[c]

Sharding: each core takes a W/8 = 2048-column slice (all 16 n, all 80 c).

The kernel is DMA-bus-bound: each SDMA engine moves read+write bytes
through a ~27.5 GB/s internal bus, so total bytes is the lever.  All
inputs are pre-cast to fp8e4m3 on the host, and x^2 is precomputed on
the host and shipped interleaved with x ([x_n ; x_n^2] = 128 rows per n,
one DMA each) - no on-device squares at all.  The 2e-2 rel-err budget
dwarfs the ~1e-3 this costs.  Only c2 stays fp32 (host-exact).

Per-core structure:
  - xz tiles: one [128, 2048] fp8 tile per n = [x ; x^2] stacked along
    the contraction dim, straight from HBM.
  - act tiles: [128, 2048] fp8 tiles covering 128 consecutive (n,c) rows
    of the [16*80, 2048] act slice.  The 80-vs-128 misalignment is
    handled per tile by 1-3 "n-runs": each run gets its own K=128 matmul
    whose [128, 128] lhsT block is the [-2c ; 1] weights zero-padded so
    the run lands on its partition range (PSUM base partitions must be
    quadrant-aligned), accumulating into the same PSUM bank.
  - One fused DVE scalar_tensor_tensor per [128, 1024] PSUM tile computes
    (dist' + c2) * act and row-sums into a column of a [128, 20]
    accumulator (c2 mapped per (tile, partition) on the host).
  - Tail: reduce the accumulator, ones^T @ racc -> [1,1] on PE.
    Host sums the 8 per-core partials and applies the clip.
  - All DMAs go through the sync HWDGE queue in exact consumption order
    (single FIFO = arrival order matches compute order); tile pools are
    sized so no buffer is ever recycled and every DMA issues with no
    waits.  x tiles are prefetched one act-tile ahead and the first/last
    act tiles are col-sliced to shrink pipeline fill and drain.
"""

import os
import sys

import numpy as np

for _p in ("/opt/trn_rl_repo",):
    if _p not in sys.path and os.path.isdir(_p):
        sys.path.insert(0, _p)

N, D, C, W = 16, 64, 80, 16384
NCORES = 8
WC = W // NCORES  # 2048 columns per core
NT = (N * C) // 128  # 10 act tiles of 128 rows
SUB = 1024  # free-dim per PSUM tile / fused DVE op
MMN = 512  # matmul free dim (one PSUM bank)
NACC = 2 * NT  # 20 accumulator columns

_CACHE = {}


def _runs():
    """Per act tile t: list of (p0, n, c0, L) n-runs covering its 128 rows."""
    out = []
    for t in range(NT):
        r0 = t * 128
        runs = []
        r = r0
        while r < r0 + 128:
            n, c0 = divmod(r, C)
            L = min(C - c0, r0 + 128 - r)
            runs.append((r - r0, n, c0, L))
            r += L
        out.append(runs)
    return out


def _build_bass():
    import concourse.bacc as bacc
    import concourse.tile as tile
    from concourse import mybir

    fp32 = mybir.dt.float32
    fp8 = mybir.dt.float8e4
    Alu = mybir.AluOpType

    nc = bacc.Bacc("TRN2", target_bir_lowering=False)

    from contextlib import ExitStack

    RUNS = _runs()
    NRUNS = sum(len(r) for r in RUNS)  # 24

    # xz rows n*128..(n+1)*128 = [x_n (64 rows) ; x_n^2 (64 rows)]
    xz = nc.dram_tensor("xz", [N * 128, WC], fp8, kind="ExternalInput")
    acts = nc.dram_tensor("acts", [N * C, WC], fp8, kind="ExternalInput")
    # wR: one zero-padded [128, 128] weight block per n-run (PSUM base
    # partition must be quadrant-aligned, so every matmul spans all 128
    # output partitions and runs accumulate into the same bank).
    wR = nc.dram_tensor("wR", [128, NRUNS * 128], fp8, kind="ExternalInput")
    wtc = nc.dram_tensor("wtc", [128, NT + 1], fp32, kind="ExternalInput")
    out = nc.dram_tensor("out", [1, 1], fp32, kind="ExternalOutput")

    with tile.TileContext(nc) as tc, ExitStack() as ctx:
        consts = ctx.enter_context(tc.tile_pool(name="consts", bufs=1))
        xpool = ctx.enter_context(tc.tile_pool(name="xpool", bufs=N))
        apool = ctx.enter_context(tc.tile_pool(name="apool", bufs=2))
        appool = ctx.enter_context(tc.tile_pool(name="appool", bufs=(NT - 2) // 2))
        spool = ctx.enter_context(tc.tile_pool(name="spool", bufs=2))
        rpool = ctx.enter_context(tc.tile_pool(name="rpool", bufs=1))
        opool = ctx.enter_context(tc.tile_pool(name="opool", bufs=1))
        pdist = ctx.enter_context(tc.tile_pool(name="pdist", bufs=3, space="PSUM"))
        psmall = ctx.enter_context(tc.tile_pool(name="psmall", bufs=1, space="PSUM"))

        racc_all = rpool.tile([128, NACC], fp32)

        xx = {}

        def load_x(n):
            t_ = xpool.tile([128, WC], fp8, tag="xx")
            nslc = 2 if n < 2 else 1
            for j in range(nslc):
                s0, s1 = j * (WC // nslc), (j + 1) * (WC // nslc)
                nc.sync.dma_start(
                    out=t_[:, s0:s1], in_=xz[n * 128 : (n + 1) * 128, s0:s1]
                )
            xx[n] = t_

        at = {}  # t -> (tile, col offset of t within the tile)

        def load_act(t):
            if t in (0, NT - 1):
                # single tile, col-sliced (fast pipeline fill / drain)
                a_ = apool.tile([128, WC], fp8, tag="at")
                nslc = 4 if t == NT - 1 else 2
                for j in range(nslc):
                    s0, s1 = j * (WC // nslc), (j + 1) * (WC // nslc)
                    nc.sync.dma_start(
                        out=a_[:, s0:s1], in_=acts[t * 128 : (t + 1) * 128, s0:s1]
                    )
                at[t] = (a_, 0)
            else:
                # paired tiles (t, t+1) in one [128, 2*WC] DMA: halves the
                # HWDGE dispatch count on the sync sequencer
                a_ = appool.tile([128, 2 * WC], fp8, tag="atp")
                src = acts[t * 128 : (t + 2) * 128, :].rearrange(
                    "(j p) w -> p j w", j=2
                )
                nc.sync.dma_start(out=a_[:], in_=src)
                at[t] = (a_, 0)
                at[t + 1] = (a_, WC)

        iacc = 0
        ri0 = 0  # index of tile t's first run within the flat run list
        for t in range(NT):
            # x for this tile AND the next (prefetch): keeps the matmuls of
            # the last tiles off the post-DMA drain path.
            for tt in (t, t + 1):
                if tt < NT:
                    for (_p0, n, _c0, _L) in RUNS[tt]:
                        if n not in xx:
                            load_x(n)
            if t not in at:
                load_act(t)
            if t == 0:
                # consts after the first x/act dispatches: they are only
                # needed once the first matmul's rhs has arrived
                wR_t = consts.tile([128, NRUNS * 128], fp8)
                nc.sync.dma_start(out=wR_t[:], in_=wR[:, :])
                wtc_t = consts.tile([128, NT + 1], fp32)
                nc.sync.dma_start(out=wtc_t[:], in_=wtc[:, :])
            nrun = len(RUNS[t])
            for si in range(WC // SUB):
                pd = pdist.tile([128, SUB], fp32, tag="pd")
                for mi in range(SUB // MMN):
                    col = si * SUB + mi * MMN
                    for k, (_p0, n, _c0, _L) in enumerate(RUNS[t]):
                        ri = ri0 + k
                        nc.tensor.matmul(
                            pd[:, mi * MMN : (mi + 1) * MMN],
                            wR_t[:, ri * 128 : (ri + 1) * 128],
                            xx[n][:, col : col + MMN],
                            start=(k == 0),
                            stop=(k == nrun - 1),
                        )
                a_, aoff = at[t]
                scr = spool.tile([128, SUB], fp32, tag="scr")
                nc.vector.scalar_tensor_tensor(
                    out=scr[:],
                    in0=pd[:],
                    scalar=wtc_t[:, t : t + 1],
                    in1=a_[:, aoff + si * SUB : aoff + (si + 1) * SUB],
                    op0=Alu.add,
                    op1=Alu.mult,
                    accum_out=racc_all[:, iacc : iacc + 1],
                )
                iacc += 1
            ri0 += nrun

        # loss_core = ones^T @ (row-sums of racc_all)
        racc = opool.tile([128, 1], fp32, tag="racc")
        nc.vector.tensor_reduce(
            out=racc[:], in_=racc_all[:], axis=mybir.AxisListType.X, op=Alu.add
        )
        pfin = psmall.tile([1, 1], fp32)
        nc.tensor.matmul(pfin[:], wtc_t[:, NT : NT + 1], racc[:], start=True, stop=True)
        osb = opool.tile([1, 1], fp32, tag="osb")
        nc.vector.tensor_copy(osb[:], pfin[:])
        nc.sync.dma_start(out=out[:, :], in_=osb[:])

    nc.compile()
    return nc


def _get_nc():
    if "nc" not in _CACHE:
        _CACHE["nc"] = _build_bass()
    return _CACHE["nc"]


def prepare_in_maps(x, c, act):
    import ml_dtypes

    fp8 = ml_dtypes.float8_e4m3fn
    x = np.ascontiguousarray(np.asarray(x), dtype=np.float32)
    c = np.ascontiguousarray(np.asarray(c), dtype=np.float32)
    act = np.ascontiguousarray(np.asarray(act), dtype=np.float32)
    assert x.shape == (N, D, W) and c.shape == (D, C) and act.shape == (N, C, W)

    x8 = x.astype(fp8)  # [N, D, W]
    xsq8 = (x8.astype(np.float32) ** 2).astype(fp8)
    # xz rows n*128..(n+1)*128 = [x_n ; x_n^2]
    xzh = np.concatenate([x8.reshape(N, D, W), xsq8.reshape(N, D, W)], axis=1)
    ab = act.astype(fp8)
    c2 = np.sum(c * c, axis=0, dtype=np.float32)  # [C]
    ones_dc = np.ones((D, C), dtype=np.float32)
    wA = np.concatenate([-2.0 * c, ones_dc], axis=0)  # [128, C]
    # Zero-padded per-run weight blocks (see wR comment in _build_bass).
    RUNS = _runs()
    blocks = []
    for t in range(NT):
        for (p0, n, c0, L) in RUNS[t]:
            blk = np.zeros((128, 128), dtype=np.float32)
            blk[:, p0 : p0 + L] = wA[:, c0 : c0 + L]
            blocks.append(blk)
    wRh = np.ascontiguousarray(np.concatenate(blocks, axis=1), dtype=fp8)
    # wtc[p, t] = c2[(t*128+p) % C] for act tile t; last col = ones.
    wtc = np.ones((128, NT + 1), dtype=np.float32)
    p = np.arange(128)
    for t in range(NT):
        wtc[:, t] = c2[(t * 128 + p) % C]
    wtc = np.ascontiguousarray(wtc)

    in_maps = []
    for k in range(NCORES):
        sl = slice(k * WC, (k + 1) * WC)
        in_maps.append(
            {
                "xz": np.ascontiguousarray(xzh[:, :, sl]).reshape(N * 128, WC),
                "acts": np.ascontiguousarray(ab[:, :, sl]).reshape(N * C, WC),
                "wR": wRh,
                "wtc": wtc,
            }
        )
    return in_maps


def kernel(x, c, act):
    from concourse.bass_utils import run_bass_kernel_spmd

    in_maps = prepare_in_maps(x, c, act)
    res = run_bass_kernel_spmd(_get_nc(), in_maps, core_ids=list(range(NCORES)))
    total = np.float32(0.0)
    for r in res.results:
        total = np.float32(total + np.float32(r["out"][0, 0]))
    return np.maximum(np.float32(total), np.float32(1e-6))


# revision 17
# speedup vs baseline: 1.5773x; 1.0938x over previous
"""CenterLoss kernel for Trainium2 (8 NeuronCores, data-parallel over W).

loss = sum_{n,c,w} act[n,c,w] * dist[n,c,w],  clipped at 1e-6, where
  dist[n,c,w] = ||x[n,:,w] - ctr[:,c]||^2 = x2[n,w] - 2*xc[n,c,w] + c2[c]

Sharding: each core takes a W/8 = 2048-column slice (all 16 n, all 80 c).

The kernel is DMA-bus-bound: each SDMA engine moves read+write bytes
through a shared internal bus, so total bytes is the lever.  All inputs
are pre-cast to fp8e4m3 on the host, and x^2 is precomputed on the host
and shipped interleaved with x - no on-device squares at all.  The 2e-2
rel-err budget dwarfs the ~4e-3 this costs.  c2 stays fp32 (host-exact).

Per-core structure:
  - xz pair tiles: one [128, 2*2048] fp8 tile per (even n, odd n) pair;
    cols [j*2048, (j+1)*2048) hold [x_n ; x_n^2] (128 rows) for n=2j+...,
    loaded with a single rearranged DMA of 256 consecutive HBM rows.
  - act tiles: [128, 2048] fp8 tiles covering 128 consecutive (n,c) rows
    of the [16*80, 2048] act slice (middle tiles DMA'd in [128, 2*2048]
    pairs).  The 80-vs-128 misalignment is handled per tile by 1-3
    "n-runs"; each run's lhsT is a [128, 128] block of the [-2c ; 1]
    weights zero-padded to its partition range (PSUM base partitions
    must be quadrant-aligned), accumulating into the same PSUM bank.
    Runs whose two n live in the same xz pair tile are fused into ONE
    fp8 DoubleRow matmul (2x PE throughput: contracts both K=128 halves
    in one pass).
  - One fused DVE scalar_tensor_tensor per [128, 1024] PSUM tile computes
    (dist' + c2) * act and row-sums into a column of a [128, 20]
    accumulator (c2 mapped per (tile, partition) on the host).
  - Tail: reduce the accumulator, ones^T @ racc -> [1,1] on PE.
    Host sums the 8 per-core partials and applies the clip.
  - All DMAs go through the sync HWDGE queue in exact consumption order
    (single FIFO = arrival order matches compute order); tile pools are
    sized so no buffer is ever recycled and every DMA issues with no
    waits.  x pairs are prefetched one act-tile ahead and the first/last
    act tiles are col-sliced to shrink pipeline fill and drain.
"""

import os
import sys

import numpy as np

for _p in ("/opt/trn_rl_repo",):
    if _p not in sys.path and os.path.isdir(_p):
        sys.path.insert(0, _p)

N, D, C, W = 16, 64, 80, 16384
NCORES = 8
WC = W // NCORES  # 2048 columns per core
NT = (N * C) // 128  # 10 act tiles of 128 rows
SUB = 1024  # free-dim per PSUM tile / fused DVE op
MMN = 512  # matmul free dim (one PSUM bank)
NACC = 2 * NT  # 20 accumulator columns

_CACHE = {}


def _runs():
    """Per act tile t: list of (p0, n, c0, L) n-runs covering its 128 rows."""
    out = []
    for t in range(NT):
        r0 = t * 128
        runs = []
        r = r0
        while r < r0 + 128:
            n, c0 = divmod(r, C)
            L = min(C - c0, r0 + 128 - r)
            runs.append((r - r0, n, c0, L))
            r += L
        out.append(runs)
    return out


def _mm_plan():
    """Per tile t: list of matmul groups, each
    ('dr', wcol, j, 256) fusing two runs from xz pair j via DoubleRow, or
    ('s', wcol, n, 128) for a single run.  wcol = column offset into wR.
    """
    plan = []
    wcol = 0
    for t, runs in enumerate(_runs()):
        groups = []
        i = 0
        while i < len(runs):
            if (
                i + 1 < len(runs)
                and runs[i][1] % 2 == 0
                and runs[i + 1][1] == runs[i][1] + 1
            ):
                groups.append(("dr", wcol, runs[i][1] // 2, (runs[i], runs[i + 1])))
                wcol += 256
                i += 2
            else:
                groups.append(("s", wcol, runs[i][1], (runs[i],)))
                wcol += 128
                i += 1
        plan.append(groups)
    return plan, wcol


def _build_bass():
    import concourse.bacc as bacc
    import concourse.tile as tile
    from concourse import mybir

    fp32 = mybir.dt.float32
    fp8 = mybir.dt.float8e4
    Alu = mybir.AluOpType
    DR = mybir.MatmulPerfMode.DoubleRow

    nc = bacc.Bacc("TRN2", target_bir_lowering=False)

    from contextlib import ExitStack

    RUNS = _runs()
    PLAN, WRC = _mm_plan()

    # xz rows n*128..(n+1)*128 = [x_n (64 rows) ; x_n^2 (64 rows)]
    xz = nc.dram_tensor("xz", [N * 128, WC], fp8, kind="ExternalInput")
    acts = nc.dram_tensor("acts", [N * C, WC], fp8, kind="ExternalInput")
    wR = nc.dram_tensor("wR", [128, WRC], fp8, kind="ExternalInput")
    wtc = nc.dram_tensor("wtc", [128, NT + 1], fp32, kind="ExternalInput")
    out = nc.dram_tensor("out", [1, 1], fp32, kind="ExternalOutput")

    with tile.TileContext(nc) as tc, ExitStack() as ctx:
        consts = ctx.enter_context(tc.tile_pool(name="consts", bufs=1))
        xpool = ctx.enter_context(tc.tile_pool(name="xpool", bufs=N // 2))
        apool = ctx.enter_context(tc.tile_pool(name="apool", bufs=2))
        appool = ctx.enter_context(tc.tile_pool(name="appool", bufs=(NT - 2) // 2))
        spool = ctx.enter_context(tc.tile_pool(name="spool", bufs=2))
        rpool = ctx.enter_context(tc.tile_pool(name="rpool", bufs=1))
        opool = ctx.enter_context(tc.tile_pool(name="opool", bufs=1))
        pdist = ctx.enter_context(tc.tile_pool(name="pdist", bufs=3, space="PSUM"))
        psmall = ctx.enter_context(tc.tile_pool(name="psmall", bufs=1, space="PSUM"))

        racc_all = rpool.tile([128, NACC], fp32)

        xx = {}  # pair index j -> [128, 2*WC] tile

        def load_xpair(j):
            t_ = xpool.tile([128, 2 * WC], fp8, tag="xx")
            src = xz[j * 256 : (j + 1) * 256, :].rearrange("(i p) w -> p i w", i=2)
            nc.sync.dma_start(out=t_[:], in_=src)
            xx[j] = t_

        at = {}  # t -> (tile, col offset of t within the tile)

        def load_act(t):
            if t in (0, NT - 1):
                # single tile, col-sliced (fast pipeline fill / drain)
                a_ = apool.tile([128, WC], fp8, tag="at")
                nslc = 4 if t == NT - 1 else 2
                for j in range(nslc):
                    s0, s1 = j * (WC // nslc), (j + 1) * (WC // nslc)
                    nc.sync.dma_start(
                        out=a_[:, s0:s1], in_=acts[t * 128 : (t + 1) * 128, s0:s1]
                    )
                at[t] = (a_, 0)
            else:
                # paired tiles (t, t+1) in one [128, 2*WC] DMA
                a_ = appool.tile([128, 2 * WC], fp8, tag="atp")
                src = acts[t * 128 : (t + 2) * 128, :].rearrange(
                    "(j p) w -> p j w", j=2
                )
                nc.sync.dma_start(out=a_[:], in_=src)
                at[t] = (a_, 0)
                at[t + 1] = (a_, WC)

        def need_x(t):
            for (_p0, n, _c0, _L) in RUNS[t]:
                if n // 2 not in xx:
                    load_xpair(n // 2)

        iacc = 0
        for t in range(NT):
            need_x(t)
            if t not in at:
                load_act(t)
            if t == 0:
                wR_t = consts.tile([128, WRC], fp8)
                nc.sync.dma_start(out=wR_t[:], in_=wR[:, :])
                wtc_t = consts.tile([128, NT + 1], fp32)
                nc.sync.dma_start(out=wtc_t[:], in_=wtc[:, :])
            if t + 1 < NT:
                need_x(t + 1)  # prefetch: keeps late matmuls off the drain
            ngrp = len(PLAN[t])
            for si in range(WC // SUB):
                pd = pdist.tile([128, SUB], fp32, tag="pd")
                for mi in range(SUB // MMN):
                    col = si * SUB + mi * MMN
                    for k, (kind, wcol, idx, _rr) in enumerate(PLAN[t]):
                        st, sp = (k == 0), (k == ngrp - 1)
                        if kind == "dr":
                            rhs = xx[idx][:, :].rearrange(
                                "p (i w) -> p i w", i=2
                            )[:, :, col : col + MMN]
                            lhsT = wR_t[:, wcol : wcol + 256].rearrange(
                                "p (i m) -> p i m", i=2
                            )
                            nc.tensor.matmul(
                                pd[:, mi * MMN : (mi + 1) * MMN],
                                lhsT,
                                rhs,
                                start=st,
                                stop=sp,
                                perf_mode=DR,
                            )
                        else:
                            n = idx
                            xoff = (n % 2) * WC
                            nc.tensor.matmul(
                                pd[:, mi * MMN : (mi + 1) * MMN],
                                wR_t[:, wcol : wcol + 128],
                                xx[n // 2][:, xoff + col : xoff + col + MMN],
                                start=st,
                                stop=sp,
                            )
                a_, aoff = at[t]
                scr = spool.tile([128, SUB], fp32, tag="scr")
                nc.vector.scalar_tensor_tensor(
                    out=scr[:],
                    in0=pd[:],
                    scalar=wtc_t[:, t : t + 1],
                    in1=a_[:, aoff + si * SUB : aoff + (si + 1) * SUB],
                    op0=Alu.add,
                    op1=Alu.mult,
                    accum_out=racc_all[:, iacc : iacc + 1],
                )
                iacc += 1

        # loss_core = ones^T @ (row-sums of racc_all)
        racc = opool.tile([128, 1], fp32, tag="racc")
        nc.vector.tensor_reduce(
            out=racc[:], in_=racc_all[:], axis=mybir.AxisListType.X, op=Alu.add
        )
        pfin = psmall.tile([1, 1], fp32)
        nc.tensor.matmul(pfin[:], wtc_t[:, NT : NT + 1], racc[:], start=True, stop=True)
        osb = opool.tile([1, 1], fp32, tag="osb")
        nc.vector.tensor_copy(osb[:], pfin[:])
        nc.sync.dma_start(out=out[:, :], in_=osb[:])

    nc.compile()
    return nc


def _get_nc():
    if "nc" not in _CACHE:
        _CACHE["nc"] = _build_bass()
    return _CACHE["nc"]


def prepare_in_maps(x, c, act):
    import ml_dtypes

    fp8 = ml_dtypes.float8_e4m3fn
    x = np.ascontiguousarray(np.asarray(x), dtype=np.float32)
    c = np.ascontiguousarray(np.asarray(c), dtype=np.float32)
    act = np.ascontiguousarray(np.asarray(act), dtype=np.float32)
    assert x.shape == (N, D, W) and c.shape == (D, C) and act.shape == (N, C, W)

    x8 = x.astype(fp8)  # [N, D, W]
    xsq8 = (x8.astype(np.float32) ** 2).astype(fp8)
    # xz rows n*128..(n+1)*128 = [x_n ; x_n^2]
    xzh = np.concatenate([x8.reshape(N, D, W), xsq8.reshape(N, D, W)], axis=1)
    ab = act.astype(fp8)
    c2 = np.sum(c * c, axis=0, dtype=np.float32)  # [C]
    ones_dc = np.ones((D, C), dtype=np.float32)
    wA = np.concatenate([-2.0 * c, ones_dc], axis=0)  # [128, C]

    def blk(p0, c0, L):
        b = np.zeros((128, 128), dtype=np.float32)
        b[:, p0 : p0 + L] = wA[:, c0 : c0 + L]
        return b

    PLAN, WRC = _mm_plan()
    cols = []
    for t in range(NT):
        for (kind, _wcol, _idx, rr) in PLAN[t]:
            for (p0, _n, c0, L) in rr:
                cols.append(blk(p0, c0, L))
    wRh = np.ascontiguousarray(np.concatenate(cols, axis=1), dtype=fp8)
    assert wRh.shape == (128, WRC)
    # wtc[p, t] = c2[(t*128+p) % C] for act tile t; last col = ones.
    wtc = np.ones((128, NT + 1), dtype=np.float32)
    p = np.arange(128)
    for t in range(NT):
        wtc[:, t] = c2[(t * 128 + p) % C]
    wtc = np.ascontiguousarray(wtc)

    in_maps = []
    for k in range(NCORES):
        sl = slice(k * WC, (k + 1) * WC)
        in_maps.append(
            {
                "xz": np.ascontiguousarray(xzh[:, :, sl]).reshape(N * 128, WC),
                "acts": np.ascontiguousarray(ab[:, :, sl]).reshape(N * C, WC),
                "wR": wRh,
                "wtc": wtc,
            }
        )
    return in_maps


def kernel(x, c, act):
    from concourse.bass_utils import run_bass_kernel_spmd

    in_maps = prepare_in_maps(x, c, act)
    res = run_bass_kernel_spmd(_get_nc(), in_maps, core_ids=list(range(NCORES)))
    total = np.float32(0.0)
    for r in res.results:
        total = np.float32(total + np.float32(r["out"][0, 0]))
    return np.maximum(np.float32(total), np.float32(1e-6))


# revision 24
# speedup vs baseline: 1.5967x; 1.0123x over previous
"""CenterLoss kernel for Trainium2 (8 NeuronCores, data-parallel over W).

loss = sum_{n,c,w} act[n,c,w] * dist[n,c,w],  clipped at 1e-6, where
  dist[n,c,w] = ||x[n,:,w] - ctr[:,c]||^2 = x2[n,w] - 2*xc[n,c,w] + c2[c]

Sharding: each core takes a W/8 = 2048-column slice (all 16 n, all 80 c).

The kernel is DMA-bus-bound: each SDMA engine moves read+write bytes
through a shared internal bus, so total bytes is the lever.  All inputs
are pre-cast to fp8e4m3 on the host, and x^2 is precomputed on the host
and shipped interleaved with x - no on-device squares at all.  The 2e-2
rel-err budget dwarfs the ~4e-3 this costs.  c2 stays fp32 (host-exact).

Per-core structure:
  - xz pair tiles: one [128, 2*2048] fp8 tile per (even n, odd n) pair;
    cols [j*2048, (j+1)*2048) hold [x_n ; x_n^2] (128 rows) for n=2j+...,
    loaded with a single rearranged DMA of 256 consecutive HBM rows.
  - act tiles: [128, 2048] fp8 tiles covering 128 consecutive (n,c) rows
    of the [16*80, 2048] act slice (middle tiles DMA'd in [128, 2*2048]
    pairs).  The 80-vs-128 misalignment is handled per tile by 1-3
    "n-runs"; each run's lhsT is a [128, 128] block of the [-2c ; 1]
    weights zero-padded to its partition range (PSUM base partitions
    must be quadrant-aligned), accumulating into the same PSUM bank.
    Runs whose two n live in the same xz pair tile are fused into ONE
    fp8 DoubleRow matmul (2x PE throughput: contracts both K=128 halves
    in one pass).
  - One fused DVE scalar_tensor_tensor per [128, 1024] PSUM tile computes
    (dist' + c2) * act and row-sums into a column of a [128, 20]
    accumulator (c2 mapped per (tile, partition) on the host).
  - Tail: reduce the accumulator, ones^T @ racc -> [1,1] on PE.
    Host sums the 8 per-core partials and applies the clip.
  - All DMAs go through the sync HWDGE queue in exact consumption order
    (single FIFO = arrival order matches compute order); tile pools are
    sized so no buffer is ever recycled and every DMA issues with no
    waits.  x pairs are prefetched one act-tile ahead and the first/last
    act tiles are col-sliced to shrink pipeline fill and drain.
"""

import os
import sys

import numpy as np

for _p in ("/opt/trn_rl_repo",):
    if _p not in sys.path and os.path.isdir(_p):
        sys.path.insert(0, _p)

N, D, C, W = 16, 64, 80, 16384
NCORES = 8
WC = W // NCORES  # 2048 columns per core
NT = (N * C) // 128  # 10 act tiles of 128 rows
SUB = 2048  # free-dim per PSUM tile / fused DVE op (4 PSUM banks)
MMN = 512  # matmul free dim (one PSUM bank)
NACC = (WC // SUB) * NT  # 10 accumulator columns

_CACHE = {}


def _runs():
    """Per act tile t: list of (p0, n, c0, L) n-runs covering its 128 rows."""
    out = []
    for t in range(NT):
        r0 = t * 128
        runs = []
        r = r0
        while r < r0 + 128:
            n, c0 = divmod(r, C)
            L = min(C - c0, r0 + 128 - r)
            runs.append((r - r0, n, c0, L))
            r += L
        out.append(runs)
    return out


def _mm_plan():
    """Per tile t: list of matmul groups, each
    ('dr', wcol, j, 256) fusing two runs from xz pair j via DoubleRow, or
    ('s', wcol, n, 128) for a single run.  wcol = column offset into wR.
    """
    plan = []
    wcol = 0
    for t, runs in enumerate(_runs()):
        groups = []
        i = 0
        while i < len(runs):
            if (
                i + 1 < len(runs)
                and runs[i][1] % 2 == 0
                and runs[i + 1][1] == runs[i][1] + 1
            ):
                groups.append(("dr", wcol, runs[i][1] // 2, (runs[i], runs[i + 1])))
                wcol += 256
                i += 2
            else:
                groups.append(("s", wcol, runs[i][1], (runs[i],)))
                wcol += 128
                i += 1
        plan.append(groups)
    return plan, wcol


def _build_bass():
    import concourse.bacc as bacc
    import concourse.tile as tile
    from concourse import mybir

    fp32 = mybir.dt.float32
    fp8 = mybir.dt.float8e4
    Alu = mybir.AluOpType
    DR = mybir.MatmulPerfMode.DoubleRow

    nc = bacc.Bacc("TRN2", target_bir_lowering=False)

    from contextlib import ExitStack

    RUNS = _runs()
    PLAN, WRC = _mm_plan()

    # xz rows n*128..(n+1)*128 = [x_n (64 rows) ; x_n^2 (64 rows)]
    xz = nc.dram_tensor("xz", [N * 128, WC], fp8, kind="ExternalInput")
    acts = nc.dram_tensor("acts", [N * C, WC], fp8, kind="ExternalInput")
    wR = nc.dram_tensor("wR", [128, WRC], fp8, kind="ExternalInput")
    wtc = nc.dram_tensor("wtc", [128, NT + 1], fp32, kind="ExternalInput")
    # per-core output: the [128, NACC] accumulator; host does the final sums
    out = nc.dram_tensor("out", [128, NACC], fp32, kind="ExternalOutput")

    with tile.TileContext(nc) as tc, ExitStack() as ctx:
        consts = ctx.enter_context(tc.tile_pool(name="consts", bufs=1))
        xpool = ctx.enter_context(tc.tile_pool(name="xpool", bufs=N // 2))
        apool = ctx.enter_context(tc.tile_pool(name="apool", bufs=2))
        appool = ctx.enter_context(tc.tile_pool(name="appool", bufs=(NT - 2) // 2))
        spool = ctx.enter_context(tc.tile_pool(name="spool", bufs=2))
        rpool = ctx.enter_context(tc.tile_pool(name="rpool", bufs=1))
        pdist = ctx.enter_context(tc.tile_pool(name="pdist", bufs=2, space="PSUM"))

        racc_all = rpool.tile([128, NACC], fp32)

        xx = {}  # pair index j -> [128, 2*WC] tile

        def load_xpair(j):
            t_ = xpool.tile([128, 2 * WC], fp8, tag="xx")
            src = xz[j * 256 : (j + 1) * 256, :].rearrange("(i p) w -> p i w", i=2)
            nc.sync.dma_start(out=t_[:], in_=src)
            xx[j] = t_

        at = {}  # t -> (tile, col offset of t within the tile)

        def load_act(t):
            if t in (0, NT - 1):
                # single tile, col-sliced (fast pipeline fill / drain)
                a_ = apool.tile([128, WC], fp8, tag="at")
                nslc = 4 if t == NT - 1 else 2
                for j in range(nslc):
                    s0, s1 = j * (WC // nslc), (j + 1) * (WC // nslc)
                    nc.sync.dma_start(
                        out=a_[:, s0:s1], in_=acts[t * 128 : (t + 1) * 128, s0:s1]
                    )
                at[t] = (a_, 0)
            else:
                # paired tiles (t, t+1) in one [128, 2*WC] DMA
                a_ = appool.tile([128, 2 * WC], fp8, tag="atp")
                src = acts[t * 128 : (t + 2) * 128, :].rearrange(
                    "(j p) w -> p j w", j=2
                )
                nc.sync.dma_start(out=a_[:], in_=src)
                at[t] = (a_, 0)
                at[t + 1] = (a_, WC)

        def need_x(t):
            for (_p0, n, _c0, _L) in RUNS[t]:
                if n // 2 not in xx:
                    load_xpair(n // 2)

        iacc = 0
        for t in range(NT):
            if t == 0:
                # first matmul needs xz pair 0 + wR; first DVE op also needs
                # wtc + act0's first half — keep exactly these at the front
                need_x(0)
                wR_t = consts.tile([128, WRC], fp8)
                nc.sync.dma_start(out=wR_t[:], in_=wR[:, :])
                wtc_t = consts.tile([128, NT + 1], fp32)
                nc.sync.dma_start(out=wtc_t[:], in_=wtc[:, :])
            need_x(t)
            if t not in at:
                load_act(t)
            if t + 1 < NT:
                need_x(t + 1)  # prefetch: keeps late matmuls off the drain
            ngrp = len(PLAN[t])
            for si in range(WC // SUB):
                pd = pdist.tile([128, SUB], fp32, tag="pd")
                for mi in range(SUB // MMN):
                    col = si * SUB + mi * MMN
                    for k, (kind, wcol, idx, _rr) in enumerate(PLAN[t]):
                        st, sp = (k == 0), (k == ngrp - 1)
                        if kind == "dr":
                            rhs = xx[idx][:, :].rearrange(
                                "p (i w) -> p i w", i=2
                            )[:, :, col : col + MMN]
                            lhsT = wR_t[:, wcol : wcol + 256].rearrange(
                                "p (i m) -> p i m", i=2
                            )
                            nc.tensor.matmul(
                                pd[:, mi * MMN : (mi + 1) * MMN],
                                lhsT,
                                rhs,
                                start=st,
                                stop=sp,
                                perf_mode=DR,
                            )
                        else:
                            n = idx
                            xoff = (n % 2) * WC
                            nc.tensor.matmul(
                                pd[:, mi * MMN : (mi + 1) * MMN],
                                wR_t[:, wcol : wcol + 128],
                                xx[n // 2][:, xoff + col : xoff + col + MMN],
                                start=st,
                                stop=sp,
                            )
                a_, aoff = at[t]
                scr = spool.tile([128, SUB], fp32, tag="scr")
                nc.vector.scalar_tensor_tensor(
                    out=scr[:],
                    in0=pd[:],
                    scalar=wtc_t[:, t : t + 1],
                    in1=a_[:, aoff + si * SUB : aoff + (si + 1) * SUB],
                    op0=Alu.add,
                    op1=Alu.mult,
                    accum_out=racc_all[:, iacc : iacc + 1],
                )
                iacc += 1

        # ship the raw accumulator; the host does the final 128x20 sum
        nc.sync.dma_start(out=out[:, :], in_=racc_all[:])

    nc.compile()
    return nc


def _get_nc():
    if "nc" not in _CACHE:
        _CACHE["nc"] = _build_bass()
    return _CACHE["nc"]


def prepare_in_maps(x, c, act):
    import ml_dtypes

    fp8 = ml_dtypes.float8_e4m3fn
    x = np.ascontiguousarray(np.asarray(x), dtype=np.float32)
    c = np.ascontiguousarray(np.asarray(c), dtype=np.float32)
    act = np.ascontiguousarray(np.asarray(act), dtype=np.float32)
    assert x.shape == (N, D, W) and c.shape == (D, C) and act.shape == (N, C, W)

    x8 = x.astype(fp8)  # [N, D, W]
    xsq8 = (x8.astype(np.float32) ** 2).astype(fp8)
    # xz rows n*128..(n+1)*128 = [x_n ; x_n^2]
    xzh = np.concatenate([x8.reshape(N, D, W), xsq8.reshape(N, D, W)], axis=1)
    ab = act.astype(fp8)
    c2 = np.sum(c * c, axis=0, dtype=np.float32)  # [C]
    ones_dc = np.ones((D, C), dtype=np.float32)
    wA = np.concatenate([-2.0 * c, ones_dc], axis=0)  # [128, C]

    def blk(p0, c0, L):
        b = np.zeros((128, 128), dtype=np.float32)
        b[:, p0 : p0 + L] = wA[:, c0 : c0 + L]
        return b

    PLAN, WRC = _mm_plan()
    cols = []
    for t in range(NT):
        for (kind, _wcol, _idx, rr) in PLAN[t]:
            for (p0, _n, c0, L) in rr:
                cols.append(blk(p0, c0, L))
    wRh = np.ascontiguousarray(np.concatenate(cols, axis=1), dtype=fp8)
    assert wRh.shape == (128, WRC)
    # wtc[p, t] = c2[(t*128+p) % C] for act tile t; last col = ones.
    wtc = np.ones((128, NT + 1), dtype=np.float32)
    p = np.arange(128)
    for t in range(NT):
        wtc[:, t] = c2[(t * 128 + p) % C]
    wtc = np.ascontiguousarray(wtc)

    in_maps = []
    for k in range(NCORES):
        sl = slice(k * WC, (k + 1) * WC)
        in_maps.append(
            {
                "xz": np.ascontiguousarray(xzh[:, :, sl]).reshape(N * 128, WC),
                "acts": np.ascontiguousarray(ab[:, :, sl]).reshape(N * C, WC),
                "wR": wRh,
                "wtc": wtc,
            }
        )
    return in_maps


def kernel(x, c, act):
    from concourse.bass_utils import run_bass_kernel_spmd

    in_maps = prepare_in_maps(x, c, act)
    res = run_bass_kernel_spmd(_get_nc(), in_maps, core_ids=list(range(NCORES)))
    total = np.float32(0.0)
    for r in res.results:
        total = np.float32(total + np.float32(np.sum(r["out"], dtype=np.float64)))
    return np.maximum(np.float32(total), np.float32(1e-6))
